# revision 9
# baseline (speedup 1.0000x reference)
"""Causal self-attention on 8 trn2 NeuronCores (bf16, interleaved pipeline).

Sharding: core c -> (batch b = c//2, head-group g = c%2 of 8 heads).
Each core computes qkv for its (batch, head-group), causal attention for
its 8 heads, and the row-slice of the output projection for its 512
channels. Host sums the two per-batch partial projections.

Design:
- all matmul operands bf16 (FWL ldweights, no narrow-N fp32r penalty,
  half DMA), PSUM accumulation fp32.
- weights DMA'd once, resident in SBUF; x streamed per token quarter.
- ScalarE does only exp; PSUM->SBUF drains on DVE.
- The attention phase A(qm) is ACT(exp)-bound: PE idles ~35% inside it.
  Those idle slots are filled by interleaving the NEXT quarter's qkv
  chains (and, during A(3), the deferred projection chains) one matmul
  at a time between score/PV blocks — a software-pipelined filler queue.
- PSUM banks (8): sw0/sw1 [128,1024] score ping-pong (4), oa0/oa1
  [65,512] attention-out accumulators (2), cs0/cs1 [128,512] filler
  chain slots for qkv/proj accumulation (2).
"""

import sys

sys.path.insert(0, "/opt/trn_rl_repo")

from collections import deque

import numpy as np
import ml_dtypes

import concourse.bass as bass
import concourse.mybir as mybir
import concourse.tile as tile
from concourse import bacc
from concourse.bass_utils import run_bass_kernel_spmd

B = 4
T = 2048
C = 1024
N_HEADS = 16
HD = 64
N_CORES = 8

H_PER_CORE = 8          # heads per core
CH = H_PER_CORE * HD    # 512 channels per core
KC = C // 128           # 8 contraction tiles over model dim
FT = CH * 2 // 128      # 8 feature tiles for q+k ([q0..q3, k0..k3])
TT = T // 128           # 16 token tiles
QM = T // 512           # 4 query macros (= quarters)
NQ = 4
CT = CH // 128          # 4 outT channel tiles
SCALE = HD ** -0.5

F32 = mybir.dt.float32
BF16 = mybir.dt.bfloat16

_CACHE = {}


def build_kernel():
    nc = bacc.Bacc(target_bir_lowering=False)

    xT = nc.dram_tensor("xT", [128, KC, T], BF16, kind="ExternalInput")
    w_qk = nc.dram_tensor("w_qk", [128, FT, KC, 128], BF16,
                          kind="ExternalInput")
    w_v = nc.dram_tensor("w_v", [128, KC, CH], BF16, kind="ExternalInput")
    w_pj = nc.dram_tensor("w_pj", [128, CT, C], BF16, kind="ExternalInput")
    masks = nc.dram_tensor("masks", [128, 4, 1024], BF16, kind="ExternalInput")
    y = nc.dram_tensor("y", [T, C], F32, kind="ExternalOutput")

    with tile.TileContext(nc) as tc:
        with (
            tc.tile_pool(name="big", bufs=1) as big,
            tc.tile_pool(name="xtp", bufs=2) as xtp,
            tc.tile_pool(name="pts", bufs=4) as pts,
            tc.tile_pool(name="sml", bufs=8) as sml,
            tc.tile_pool(name="ysb", bufs=4) as ysbp,
            tc.tile_pool(name="ps", bufs=1, space="PSUM") as ps,
        ):
            # ---- resident SBUF tensors ----
            qkT = [big.tile([128, T], BF16, tag=f"qkT{ft}", name=f"qkT{ft}")
                   for ft in range(FT)]
            vt_all = big.tile([128, TT, H_PER_CORE, HD + 1], BF16,
                              tag="vt_all", name="vt_all")
            vt = [vt_all[:, tt] for tt in range(TT)]
            outT = [big.tile([128, T], BF16, tag=f"outT{ct}", name=f"outT{ct}")
                    for ct in range(CT)]
            wq_all = big.tile([128, FT, KC, 128], BF16, tag="wq_all",
                              name="wq_all")
            wq = [wq_all[:, ft] for ft in range(FT)]
            wv_all = big.tile([128, KC, CH], BF16, tag="wv_all",
                              name="wv_all")
            wv = [wv_all[:, kc] for kc in range(KC)]
            wpj_all = big.tile([128, CT, C], BF16, tag="wpj_all",
                               name="wpj_all")
            wpj = [wpj_all[:, ct] for ct in range(CT)]
            mask_sb = big.tile([128, 4, 1024], BF16, tag="masks", name="masks")

            # PSUM: 8 banks, statically assigned.
            sw01 = [ps.tile([128, 1024], F32, tag=f"sw{i}", name=f"sw{i}")
                    for i in range(2)]
            oa = [ps.tile([HD + 1, 512], F32, tag=f"oa{i}", name=f"oa{i}")
                  for i in range(2)]
            cs = [ps.tile([128, 512], F32, tag=f"cs{i}", name=f"cs{i}")
                  for i in range(2)]

            # ---- input DMAs: x quarter 0 first (unblocks Q(0)), then
            # weights in first-use order; wpj (needed last) at the end.
            xts_all = {}

            def load_x(tq, split=False):
                xt = xtp.tile([128, KC, 512], BF16, tag="xt", name="xt")
                if split:
                    nc.sync.dma_start(
                        out=xt[:, 0:4], in_=xT[:, 0:4,
                                               tq * 512:(tq + 1) * 512])
                    nc.sync.dma_start(
                        out=xt[:, 4:8], in_=xT[:, 4:8,
                                               tq * 512:(tq + 1) * 512])
                else:
                    nc.sync.dma_start(
                        out=xt, in_=xT[:, :, tq * 512:(tq + 1) * 512])
                xts_all[tq] = [xt[:, kc] for kc in range(KC)]

            # warmup source filled on-chip: no DMA dependency, so the PE
            # warms its HAM busy-window while the first inputs stream in.
            wu_t = big.tile([128, 512], BF16, tag="wu", name="wu")
            nc.gpsimd.memset(wu_t[:], 0.03125)
            for wu in range(20):
                nc.tensor.matmul(
                    cs[wu % 2][:], wu_t[:, 0:128], wu_t[:],
                    start=True, stop=True)

            load_x(0, split=True)
            for fc in range(4):       # wq in per-pair chunks, priority order
                nc.sync.dma_start(out=wq_all[:, 2 * fc:2 * fc + 2],
                                  in_=w_qk[:, 2 * fc:2 * fc + 2])
            nc.sync.dma_start(out=mask_sb, in_=masks[:])
            nc.sync.dma_start(out=wv_all[:, 0:4], in_=w_v[:, 0:4])
            nc.sync.dma_start(out=wv_all[:, 4:8], in_=w_v[:, 4:8])
            nc.gpsimd.memset(vt_all[:, :, :, HD:HD + 1], 1.0)
            nc.sync.dma_start(out=wpj_all, in_=w_pj[:])

            # ---- filler machinery: queued single-matmul steps ----
            filler = deque()
            slot_ctr = [0]

            def next_slot():
                s = cs[slot_ctr[0] % 2]
                slot_ctr[0] += 1
                return s

            def queue_qk_chain(tq, ft):
                acc = next_slot()

                def mk(kc):
                    def f():
                        nc.tensor.matmul(
                            acc[:], wq[ft][:, kc, :], xts_all[tq][kc][:],
                            start=(kc == 0), stop=(kc == KC - 1))
                        if kc == KC - 1:
                            nc.vector.tensor_copy(
                                qkT[ft][:, tq * 512:(tq + 1) * 512], acc[:])
                    return f

                for kc in range(KC):
                    filler.append(mk(kc))

            def queue_v_chain(tq, i):
                acc = next_slot()
                tt = tq * 4 + i

                def mk(kc):
                    def f():
                        nc.tensor.matmul(
                            acc[:],
                            xts_all[tq][kc][:, i * 128:(i + 1) * 128],
                            wv[kc][:],
                            start=(kc == 0), stop=(kc == KC - 1))
                        if kc == KC - 1:
                            nc.vector.tensor_copy(
                                vt[tt][:, :, 0:HD],
                                acc[:].rearrange("p (h d) -> p h d",
                                                 h=H_PER_CORE))
                    return f

                for kc in range(KC):
                    filler.append(mk(kc))

            def queue_q_phase(tq):
                load_x(tq)
                for ft in range(4):
                    queue_qk_chain(tq, ft)
                for i in range(4):
                    queue_v_chain(tq, i)
                for ft in range(4, FT):
                    queue_qk_chain(tq, ft)

            def queue_p_chain(tq, i, nf):
                acc = next_slot()
                tt = tq * 4 + i

                def mk(ct):
                    def f():
                        nc.tensor.matmul(
                            acc[:],
                            outT[ct][:, tt * 128:(tt + 1) * 128],
                            wpj[ct][:, nf * 512:(nf + 1) * 512],
                            start=(ct == 0), stop=(ct == CT - 1))
                        if ct == CT - 1:
                            ys = ysbp.tile([128, 512], F32, tag="ys",
                                           name="ys")
                            nc.vector.tensor_copy(ys[:], acc[:])
                            nc.sync.dma_start(
                                out=y[tt * 128:(tt + 1) * 128,
                                      nf * 512:(nf + 1) * 512],
                                in_=ys[:])
                    return f

                for ct in range(CT):
                    filler.append(mk(ct))

            def queue_p_phase(tq):
                for i in range(4):
                    for nf in range(2):
                        queue_p_chain(tq, i, nf)

            def emit_filler(n):
                for _ in range(n):
                    if filler:
                        filler.popleft()()

            def drain_filler():
                while filler:
                    filler.popleft()()

            # ---- Q(0): no attention to hide it under; emit directly ----
            queue_q_phase(0)
            drain_filler()

            # ---- main loop: A(tq) with fillers from Q(tq+1) / P(<3) ----
            for tq in range(NQ):
                qm = tq
                nkt = 4 * qm + 4
                if tq < 3:
                    queue_q_phase(tq + 1)
                else:
                    for ptq in range(3):
                        queue_p_phase(ptq)
                for p in range(4):          # head pair = heads 2p, 2p+1
                    qTh = qkT[2 * p]
                    kTh = qkT[2 * p + 1]
                    pts_q = []

                    def scores_block(kt):
                        j = kt - 4 * qm     # >=0 on diagonal blocks
                        o0 = max(j, 0) * 128
                        sw = sw01[kt % 2]
                        for hh in range(2):
                            nc.tensor.matmul(
                                sw[:, hh * 512 + o0:(hh + 1) * 512],
                                kTh[hh * 64:(hh + 1) * 64,
                                    kt * 128:(kt + 1) * 128],
                                qTh[hh * 64:(hh + 1) * 64,
                                    qm * 512 + o0:(qm + 1) * 512],
                                start=True, stop=True)
                        pt = pts.tile([128, 1024], BF16, tag="pT", name="pT")
                        swv = sw[:].rearrange("p (a q) -> p a q", a=2)
                        ptv = pt[:].rearrange("p (a q) -> p a q", a=2)
                        nc.scalar.activation(
                            ptv[:, :, o0:512], swv[:, :, o0:512],
                            mybir.ActivationFunctionType.Exp, scale=SCALE)
                        if j >= 0:      # diagonal block: 0/1 mask both heads
                            mv = mask_sb[:, j, :].rearrange(
                                "p (a q) -> p a q", a=2)
                            nc.vector.tensor_mul(
                                ptv[:, :, o0:512], ptv[:, :, o0:512],
                                mv[:, :, o0:512])
                        pts_q.append((pt, o0))

                    def pv_block(kt):
                        pt, o0 = pts_q[kt]
                        for hh in range(2):
                            h = 2 * p + hh
                            nc.tensor.matmul(
                                oa[hh][:, o0:512],
                                vt[kt][:, h, :],
                                pt[:, hh * 512 + o0:(hh + 1) * 512],
                                start=(kt == 0), stop=(kt == nkt - 1),
                                skip_group_check=True)

                    # one-deep software pipeline: while exp(kt) runs on
                    # ScalarE, the PE does scores(kt+1) plus fillers, so
                    # PV(kt) rarely blocks and filler ldweights stay hidden.
                    for kt in range(nkt):
                        scores_block(kt)
                        if kt > 0:
                            emit_filler(3)
                            pv_block(kt - 1)
                    emit_filler(2)
                    pv_block(nkt - 1)
                    for hh in range(2):
                        oacc = oa[hh]
                        den = sml.tile([1, 512], F32, tag="den", name="den")
                        nc.vector.tensor_copy(den[:], oacc[HD:HD + 1, 0:512])
                        rd = sml.tile([1, 512], F32, tag="rd", name="rd")
                        nc.vector.reciprocal_approx_fast(rd[:], den[:])
                        bcs = sml.tile([HD, 512], F32, tag="bcs", name="bcs")
                        nc.gpsimd.partition_broadcast(bcs[:], rd[:])
                        nc.vector.tensor_mul(
                            outT[p][hh * 64:(hh + 1) * 64,
                                    qm * 512:(qm + 1) * 512],
                            oacc[0:HD, 0:512], bcs[:])
                    emit_filler(4)
                drain_filler()

            # ---- P(3): tail projection for the last quarter ----
            queue_p_phase(3)
            drain_filler()

    nc.compile()
    return nc


def _make_masks():
    k = np.arange(128)[:, None, None]
    j = np.arange(4)[None, :, None]
    q = np.arange(512)[None, None, :]
    m = (j * 128 + k <= q)                       # [128, 4, 512]
    m2 = np.concatenate([m, m], axis=2)          # [128, 4, 1024] (both heads)
    return m2.astype(ml_dtypes.bfloat16)


def make_in_maps(x, w_qkv, w_proj):
    bf = ml_dtypes.bfloat16
    masks = _make_masks()
    in_maps = []
    for c in range(N_CORES):
        b, g = c // 2, c % 2
        # [p, kc, t]: partition p = channel-within-chunk, kc = chunk
        xTv = np.ascontiguousarray(
            x[b].T.reshape(KC, 128, T).transpose(1, 0, 2).astype(bf))
        wq_ = w_qkv[:, g * CH:(g + 1) * CH]
        wk_ = w_qkv[:, C + g * CH:C + (g + 1) * CH]
        stacked = np.concatenate([wq_, wk_], axis=1)         # [1024, 1024]
        # [p, ft, kc, f] with ft interleaved [q0,k0,q1,k1,...] so that the
        # per-pair weights arrive in the first DMA chunks
        w_qk = np.ascontiguousarray(
            stacked.reshape(KC, 128, FT, 128).transpose(1, 2, 0, 3)
            [:, [0, 4, 1, 5, 2, 6, 3, 7]].astype(bf))
        # [p, kc, ch]
        w_v = np.ascontiguousarray(
            w_qkv[:, 2 * C + g * CH:2 * C + (g + 1) * CH]
            .reshape(KC, 128, CH).transpose(1, 0, 2).astype(bf))
        # [p, ct, c]
        w_pj = np.ascontiguousarray(
            w_proj[g * CH:(g + 1) * CH, :]
            .reshape(CT, 128, C).transpose(1, 0, 2).astype(bf))
        in_maps.append({
            "xT": xTv, "w_qk": w_qk, "w_v": w_v, "w_pj": w_pj,
            "masks": masks,
        })
    return in_maps


def kernel(x, w_qkv, w_proj):
    x = np.asarray(x, dtype=np.float32)
    w_qkv = np.asarray(w_qkv, dtype=np.float32)
    w_proj = np.asarray(w_proj, dtype=np.float32)

    if "nc" not in _CACHE:
        _CACHE["nc"] = build_kernel()
    nc = _CACHE["nc"]

    in_maps = make_in_maps(x, w_qkv, w_proj)
    res = run_bass_kernel_spmd(nc, in_maps, core_ids=list(range(N_CORES)))
    _CACHE["last_result"] = res

    yout = np.empty((B, T, C), dtype=np.float32)
    for b in range(B):
        yout[b] = res.results[2 * b]["y"] + res.results[2 * b + 1]["y"]
    return yout
